# revision 30
# baseline (speedup 1.0000x reference)
"""Fixed-point attention (nn_Attention_17918603559191) on 8 TRN2 NeuronCores.

Sharding (no collectives): core c -> batch b = c//2, token-half t = c%2.
Each core computes K,V for all 2048 tokens of its batch and Q/attention/
proj for its 1024 tokens. The host rotates the token axis of x per core so
the core's q-tokens are always columns 0..1023 (identical SPMD program);
attention is invariant to permuting the key/value axis.

Numerics (validated vs reference in numpy, rel ~1.0e-2 < 2e-2 gate):
 - all matmuls fp16 operands (1 cyc/row on PE), fp32 PSUM accumulation
 - weights in natural units (no 4096 prescale)
 - q/k/v/ao/out: plain clip+round-to-fp16 cast instead of the reference's
   grid trunc (validated: adds ~1e-3 rel noise, zero systematic bias)
 - attn = floor(exp * 4096/S) kept exact-ish: exp fp16 on ACT,
   S-broadcast via ones[128,128] matmul, rb = 1/S on DVE [128,512],
   floor via (exp*4096*rb) - 0.499 -> int16 (DVE) -> fp16 (GpSimd)

Phase-2 software pipeline (per (hp,ch) iteration i):
   PE issue order: dots(i) ; S(i-1) ; av(i-2)   -- so PE never waits on
   ACT exp / DVE quantize chains. exp_b double-buffered.
"""

import sys

sys.path.insert(0, "/opt/trn_rl_repo")

import os
import numpy as np
import concourse.bass as bass
import concourse.tile as tile
from concourse import mybir, bacc
from concourse.bass_utils import run_bass_kernel_spmd

K_ACTPATCH = os.environ.get("K_ACTPATCH", "0") == "1"
K_LNEXP = os.environ.get("K_LNEXP", "0") == "1"
K_SORDER = os.environ.get("K_SORDER", "1") == "1"

F32 = mybir.dt.float32
F16 = mybir.dt.float16
I16 = mybir.dt.int16
ALU = mybir.AluOpType
AF = mybir.ActivationFunctionType

D = 1024      # model dim
M = 2048      # key/value tokens per core (full batch)
NQ = 1024     # query tokens per core
H = 16
DH = 64
HP = H // 2   # head pairs
GRID = 4096.0

_CACHED_NC = None


def _rep_free(ap, g):
    """Repeat a [P, W] AP g times along a new middle free dim (stride 0)."""
    return bass.AP(tensor=ap.tensor, offset=ap.offset,
                   ap=[ap.ap[0], [0, g], ap.ap[1]])


def _patch_act_tables(nc):
    """Make the act-table placement prefer the set containing BOTH Ln and
    Exp (natural_log_exp_and_others); the default greedy order alternates
    between exp-only and ln-bearing sets, emitting a table load per call."""
    import types
    from concourse.hw_specs import get_activation_tables
    import bass_rust as _bass_rust

    def insert_act_table_loads(self):
        has_activation = any(
            isinstance(i, mybir.InstActivation)
            for b in self.main_func.blocks
            for i in b.instructions
        )
        if not has_activation:
            return
        tables = list(get_activation_tables(self.m.arch).items())
        combined = [fns for name, fns in tables if "natural_log_exp" in name]
        used = set()
        for b in self.main_func.blocks:
            for i in b.instructions:
                if isinstance(i, mybir.InstActivation):
                    used.add(i.func)
        if combined and used <= combined[0]:
            # keep positions (set ids) but disqualify competing sets so the
            # greedy placement settles on the one combined set
            tables = [(name, fns if "natural_log_exp" in name else set())
                      for name, fns in tables]
        _bass_rust.insert_act_table_loads(self, tables)

    nc.insert_act_table_loads = types.MethodType(insert_act_table_loads, nc)


def build_kernel(reps=1):
    nc = bacc.Bacc(name="fxp_attn")
    if K_ACTPATCH:
        _patch_act_tables(nc)
    xT_e = nc.declare_dram_parameter("xT", [D, M], F32, isOutput=False)
    wqkvT_e = nc.declare_dram_parameter("wqkvT", [D, 3 * D], F32, isOutput=False)
    wprojT_e = nc.declare_dram_parameter("wprojT", [D, D], F32, isOutput=False)
    bias_e = nc.declare_dram_parameter("bias", [1, D], F32, isOutput=False)
    out_e = nc.declare_dram_parameter("out", [D, NQ], F32, isOutput=True)

    with tile.TileContext(nc) as tc:
        from contextlib import ExitStack
        with ExitStack() as ctx:
            persist = ctx.enter_context(tc.tile_pool(name="persist", bufs=1))

            # ones/4096: S-broadcast matmul directly yields S/4096, so the
            # reciprocal gives rb = 4096/S with no extra scale op.
            ones128 = persist.tile([128, 128], F16, tag="ones128")
            nc.vector.memset(ones128, 1.0 / GRID)
            bias_sb = persist.tile([128, 8], F32, tag="bias")
            nc.sync.dma_start(out=bias_sb, in_=bass.AP(
                tensor=bias_e.ap().tensor, offset=0, ap=[[1, 128], [128, 8]]))

            # persistent fp16 activations
            k_s = [persist.tile([128, M], F16, tag=f"k{s}", name=f"k{s}") for s in range(8)]
            q_s = [persist.tile([128, NQ], F16, tag=f"q{s}", name=f"q{s}") for s in range(8)]
            v_t = [persist.tile([128, D], F16, tag=f"v{t}", name=f"v{t}") for t in range(16)]
            ao_s = [persist.tile([128, NQ], F16, tag=f"ao{s}", name=f"ao{s}")
                    for s in range(8)]

            for _rep in range(reps):
                _run_phases(nc, tc, ones128, bias_sb,
                            k_s, q_s, v_t, ao_s,
                            xT_e, wqkvT_e, wprojT_e, out_e)

    nc.compile()
    return nc


def _run_phases(nc, tc, ones128, bias_sb,
                k_s, q_s, v_t, ao_s,
                xT_e, wqkvT_e, wprojT_e, out_e):
    # ---------------- Phase 0/1: load inputs, qkv matmuls ----------
    with tc.tile_pool(name="ph1", bufs=1) as ph1, \
         tc.tile_pool(name="stg", bufs=2) as stg, \
         tc.tile_pool(name="wstg", bufs=2) as wstg, \
         tc.tile_pool(name="ps1", bufs=8, space="PSUM") as ps1:

        # xT fp16, resident: [128, 8, 2048]
        xT = ph1.tile([128, 8, M], F16, tag="xT")
        for s in range(8):
            st = stg.tile([128, M], F32, tag="stg")
            nc.sync.dma_start(out=st, in_=xT_e.ap()[s * 128:(s + 1) * 128, :])
            nc.gpsimd.tensor_copy(xT[:, s, :], st)

        # w_v fp16 resident: wqkvT cols [2048:3072] -> [128, 8, 1024]
        wv = ph1.tile([128, 8, D], F16, tag="wv")
        for quart in range(4):
            st = stg.tile([128, M], F32, tag="stg")
            st3 = bass.AP(tensor=st.tensor, offset=st.offset,
                          ap=[st.ap[0], [256, 8], [1, 256]])
            nc.sync.dma_start(out=st3, in_=bass.AP(
                tensor=wqkvT_e.ap().tensor,
                offset=2 * D + quart * 256,
                ap=[[3 * D, 128], [3 * D * 128, 8], [1, 256]]))
            nc.gpsimd.tensor_copy(wv[:, :, quart * 256:(quart + 1) * 256], st3)

        def stream_w(col0, ncols):
            """DMA wqkvT[:, col0:col0+ncols] -> f16 [128, 8, ncols]."""
            st = wstg.tile([128, 8, 128], F32, tag="wstg")
            wf = wstg.tile([128, 8, 128], F16, tag="wstg16")
            nc.sync.dma_start(out=st[:, :, :ncols], in_=bass.AP(
                tensor=wqkvT_e.ap().tensor, offset=col0,
                ap=[[3 * D, 128], [3 * D * 128, 8], [1, ncols]]))
            nc.vector.tensor_scalar(wf[:, :, :ncols], st[:, :, :ncols],
                                    1.0, None, op0=ALU.mult)
            return wf

        # K: feature-major [kv-dim strip s][128, 2048]  (evac on ACT)
        # dt-major inner loops reuse each stationary wf[:, dt, :] across all
        # mc chunks (one LDWEIGHTS per dt instead of per matmul).
        for s in range(8):
            wf = stream_w(D + s * 128, 128)
            pts = [ps1.tile([128, 512], F32, tag="ps1", name=f"k{s}_{mc}")
                   for mc in range(4)]
            for dt in range(8):
                for mc in range(4):
                    nc.tensor.matmul(
                        pts[mc], lhsT=wf[:, dt, :],
                        rhs=xT[:, dt, mc * 512:(mc + 1) * 512],
                        start=(dt == 0), stop=(dt == 7))
            for mc in range(4):
                nc.scalar.copy(k_s[s][:, mc * 512:(mc + 1) * 512], pts[mc])

        # Q: feature-major, tokens 0..1023 of rotated xT  (evac on DVE)
        for s in range(8):
            wf = stream_w(s * 128, 128)
            pts = [ps1.tile([128, 512], F32, tag="ps1", name=f"q{s}_{mc}")
                   for mc in range(2)]
            for dt in range(8):
                for mc in range(2):
                    nc.tensor.matmul(
                        pts[mc], lhsT=wf[:, dt, :],
                        rhs=xT[:, dt, mc * 512:(mc + 1) * 512],
                        start=(dt == 0), stop=(dt == 7))
            for mc in range(2):
                nc.vector.tensor_scalar(q_s[s][:, mc * 512:(mc + 1) * 512],
                                        pts[mc], 1.0, None, op0=ALU.mult)

        # V: token-major [tok strip ts][128, 1024]  (evac on DVE)
        for ts in range(16):
            pts = [ps1.tile([128, 512], F32, tag="ps1", name=f"v{ts}_{cc}")
                   for cc in range(2)]
            for dt in range(8):
                for cc in range(2):
                    nc.tensor.matmul(
                        pts[cc], lhsT=xT[:, dt, ts * 128:(ts + 1) * 128],
                        rhs=wv[:, dt, cc * 512:(cc + 1) * 512],
                        start=(dt == 0), stop=(dt == 7))
            for cc in range(2):
                nc.vector.tensor_scalar(v_t[ts][:, cc * 512:(cc + 1) * 512],
                                        pts[cc], 1.0, None, op0=ALU.mult)

    # ---------------- Phase 2: attention ---------------------------
    with tc.tile_pool(name="expp", bufs=2) as expp, \
         tc.tile_pool(name="attn", bufs=2) as attnp, \
         tc.tile_pool(name="rbp", bufs=2) as rbp, \
         tc.tile_pool(name="dotp", bufs=2, space="PSUM") as dotp, \
         tc.tile_pool(name="sbp", bufs=1, space="PSUM") as sbp, \
         tc.tile_pool(name="avp", bufs=2, space="PSUM") as avp:

        ITERS = [(hp, ch) for hp in range(HP) for ch in range(2)]
        NIT = len(ITERS)

        exp_tiles = {}
        rb_tiles = {}
        af_tiles = {}
        av_tiles = {}

        def issue_dots(i):
            hp, ch = ITERS[i]
            n0 = ch * 512
            eb = expp.tile([128, 2, 16, 512], F16, tag="exp", name=f"exp{i}")
            exp_tiles[i] = eb
            for mt in range(16):
                dt_ps = dotp.tile([128, 2, 512], F32, tag="dt")
                for h in range(2):
                    p0 = h * 64
                    nc.tensor.matmul(
                        dt_ps[:, h, :],
                        lhsT=k_s[hp][p0:p0 + 64, mt * 128:(mt + 1) * 128],
                        rhs=q_s[hp][p0:p0 + 64, n0:n0 + 512],
                        start=True, stop=True,
                        tile_position=(p0, 0))
                nc.scalar.activation(eb[:, :, mt, :], dt_ps, AF.Exp,
                                     scale=0.125)

        def issue_s(i):
            eb = exp_tiles[i]
            rbs = []
            for h in range(2):
                sbc = sbp.tile([128, 512], F32, tag=f"sbc{h}")
                for mt in range(16):
                    nc.tensor.matmul(sbc, lhsT=ones128,
                                     rhs=eb[:, h, mt, :],
                                     start=(mt == 0), stop=(mt == 15))
                # rb4 = 4096/S replicated x4 (unit-stride operand for tt).
                # 1/x via exp(-ln x) on ACT: frees DVE, plenty precise.
                rb4 = rbp.tile([128, 4, 512], F16, tag=f"rb4{h}")
                if K_LNEXP:
                    lnS = rbp.tile([128, 512], F16, tag=f"ln{h}", bufs=1)
                    nc.scalar.activation(lnS, sbc, AF.Ln)
                    with nc.allow_low_precision(reason="1/S fp16 validated"):
                        nc.scalar.activation(rb4[:, 0, :], lnS, AF.Exp,
                                             scale=-1.0)
                else:
                    with nc.allow_low_precision(reason="1/S fp16 validated"):
                        nc.vector.reciprocal(rb4[:, 0, :], sbc)
                for j in range(1, 4):
                    nc.gpsimd.tensor_copy(rb4[:, j, :], rb4[:, 0, :])
                rbs.append(rb4)
            rb_tiles[i] = rbs

        def issue_quant_av(i):
            hp, ch = ITERS[i]
            n0 = ch * 512
            eb = exp_tiles.pop(i)
            rbs = rb_tiles.pop(i)
            av = avp.tile([128, 512], F32, tag="av")
            for g in range(4):
                afs = []
                for h in range(2):
                    y = attnp.tile([128, 4, 512], F16, tag="y", bufs=1)
                    nc.vector.tensor_tensor(
                        y, eb[:, h, 4 * g:4 * g + 4, :], rbs[h], op=ALU.mult)
                    ai = attnp.tile([128, 4, 512], I16, tag="ai", bufs=1)
                    nc.vector.tensor_scalar(ai, y, -0.499, None, op0=ALU.add)
                    af = attnp.tile([128, 4, 512], F16, tag="af",
                                    name=f"af{i}_{g}_{h}")
                    nc.vector.tensor_scalar(af, ai, 1.0, None, op0=ALU.mult)
                    afs.append(af)
                for sub in range(4):
                    mt = 4 * g + sub
                    for h in range(2):
                        p0 = h * 64
                        nc.tensor.matmul(
                            av[p0:p0 + 64, :],
                            lhsT=v_t[mt][:, (2 * hp + h) * 64:
                                         (2 * hp + h + 1) * 64],
                            rhs=afs[h][:, sub, :],
                            start=(mt == 0), stop=(mt == 15),
                            tile_position=(0, p0))
            # evacuate: natural units = grid/4096
            nc.vector.tensor_scalar(ao_s[hp][:, n0:n0 + 512], av,
                                    1.0 / GRID, None, op0=ALU.mult)

        # software pipeline, PE issue order per step: S(i-1); dots(i); av(i-1)
        for i in range(NIT + 1):
            if K_SORDER:
                if i >= 1:
                    issue_s(i - 1)
                if i < NIT:
                    issue_dots(i)
                if i >= 1:
                    issue_quant_av(i - 1)
            else:
                if i < NIT:
                    issue_dots(i)
                if i >= 1:
                    issue_s(i - 1)
                    issue_quant_av(i - 1)

    # ---------------- Phase 3: projection --------------------------
    with tc.tile_pool(name="ps3", bufs=4, space="PSUM") as ps3, \
         tc.tile_pool(name="wpp", bufs=1) as wpp, \
         tc.tile_pool(name="wstg3", bufs=2) as wstg3, \
         tc.tile_pool(name="outp", bufs=2) as outp:
        wp_s = []
        for s in range(8):
            st = wstg3.tile([128, D], F32, tag="stg3")
            nc.sync.dma_start(out=st, in_=wprojT_e.ap()[s * 128:(s + 1) * 128, :])
            wp = wpp.tile([128, D], F16, tag=f"wp{s}")
            nc.vector.tensor_scalar(wp, st, 1.0, None, op0=ALU.mult)
            wp_s.append(wp)
        for ds in range(8):
            pts = [ps3.tile([128, 512], F32, tag="ps3", name=f"p3_{ds}_{ch}")
                   for ch in range(2)]
            for es in range(8):
                for ch in range(2):
                    nc.tensor.matmul(
                        pts[ch], lhsT=wp_s[es][:, ds * 128:(ds + 1) * 128],
                        rhs=ao_s[es][:, ch * 512:(ch + 1) * 512],
                        start=(es == 0), stop=(es == 7))
            for ch in range(2):
                ot = outp.tile([128, 512], F32, tag="ot")
                nc.vector.tensor_scalar(ot, pts[ch], bias_sb[:, ds:ds + 1],
                                        None, op0=ALU.add)
                nc.sync.dma_start(
                    out=out_e.ap()[ds * 128:(ds + 1) * 128,
                                   ch * 512:(ch + 1) * 512],
                    in_=ot)


def _get_nc():
    global _CACHED_NC
    if _CACHED_NC is None:
        _CACHED_NC = build_kernel()
    return _CACHED_NC


def prep(inputs):
    """Build (nc, in_maps) for the 8 cores from full inputs."""
    x, w_qkv, w_proj, b_proj = (inputs["x"], inputs["w_qkv"],
                                inputs["w_proj"], inputs["b_proj"])
    nc = _get_nc()
    wqkvT = np.ascontiguousarray(w_qkv.astype(np.float32).T)
    wprojT = np.ascontiguousarray(w_proj.astype(np.float32).T)
    bias = b_proj.astype(np.float32).reshape(1, D)

    in_maps = []
    for c in range(8):
        b, t = c // 2, c % 2
        xb = x[b].astype(np.float32)
        xrot = np.concatenate([xb[t * NQ:], xb[:t * NQ]], axis=0)
        in_maps.append({
            "xT": np.ascontiguousarray(xrot.T),
            "wqkvT": wqkvT,
            "wprojT": wprojT,
            "bias": bias,
        })
    return nc, in_maps


def kernel(x, w_qkv, w_proj, b_proj, **_):
    B, N, Dm = x.shape
    assert (B, N, Dm) == (4, 2048, 1024)
    nc, in_maps = prep({"x": x, "w_qkv": w_qkv, "w_proj": w_proj,
                        "b_proj": b_proj})

    res = run_bass_kernel_spmd(nc, in_maps, list(range(8)))
    global LAST_RESULT
    LAST_RESULT = res
    out = np.empty((B, N, Dm), dtype=np.float32)
    for c in range(8):
        b, t = c // 2, c % 2
        out[b, t * NQ:(t + 1) * NQ, :] = res.results[c]["out"].T
    return out


# revision 32
# speedup vs baseline: 1.1857x; 1.1857x over previous
"""Fixed-point attention (nn_Attention_17918603559191) on 8 TRN2 NeuronCores.

Sharding (no collectives): core c -> batch b = c//2, token-half t = c%2.
Each core computes K,V for all 2048 tokens of its batch and Q/attention/
proj for its 1024 tokens. The host rotates the token axis of x per core so
the core's q-tokens are always columns 0..1023 (identical SPMD program);
attention is invariant to permuting the key/value axis.

Numerics (validated vs reference in numpy, rel ~1.0e-2 < 2e-2 gate):
 - all matmuls fp16 operands (1 cyc/row on PE), fp32 PSUM accumulation
 - weights in natural units (no 4096 prescale)
 - q/k/v/ao/out: plain clip+round-to-fp16 cast instead of the reference's
   grid trunc (validated: adds ~1e-3 rel noise, zero systematic bias)
 - attn = floor(exp * 4096/S) kept exact-ish: exp fp16 on ACT,
   S-broadcast via ones[128,128] matmul, rb = 1/S on DVE [128,512],
   floor via (exp*4096*rb) - 0.499 -> int16 (DVE) -> fp16 (GpSimd)

Phase-2 software pipeline (per (hp,ch) iteration i):
   PE issue order: dots(i) ; S(i-1) ; av(i-2)   -- so PE never waits on
   ACT exp / DVE quantize chains. exp_b double-buffered.
"""

import sys

sys.path.insert(0, "/opt/trn_rl_repo")

import os
import numpy as np
import concourse.bass as bass
import concourse.tile as tile
from concourse import mybir, bacc
from concourse.bass_utils import run_bass_kernel_spmd

K_ACTPATCH = os.environ.get("K_ACTPATCH", "0") == "1"
K_LNEXP = os.environ.get("K_LNEXP", "0") == "1"
K_SORDER = os.environ.get("K_SORDER", "1") == "1"

F32 = mybir.dt.float32
F16 = mybir.dt.float16
I16 = mybir.dt.int16
ALU = mybir.AluOpType
AF = mybir.ActivationFunctionType

D = 1024      # model dim
M = 2048      # key/value tokens per core (full batch)
NQ = 1024     # query tokens per core
H = 16
DH = 64
HP = H // 2   # head pairs
GRID = 4096.0

_CACHED_NC = None


def _rep_free(ap, g):
    """Repeat a [P, W] AP g times along a new middle free dim (stride 0)."""
    return bass.AP(tensor=ap.tensor, offset=ap.offset,
                   ap=[ap.ap[0], [0, g], ap.ap[1]])


def _patch_act_tables(nc):
    """Make the act-table placement prefer the set containing BOTH Ln and
    Exp (natural_log_exp_and_others); the default greedy order alternates
    between exp-only and ln-bearing sets, emitting a table load per call."""
    import types
    from concourse.hw_specs import get_activation_tables
    import bass_rust as _bass_rust

    def insert_act_table_loads(self):
        has_activation = any(
            isinstance(i, mybir.InstActivation)
            for b in self.main_func.blocks
            for i in b.instructions
        )
        if not has_activation:
            return
        tables = list(get_activation_tables(self.m.arch).items())
        combined = [fns for name, fns in tables if "natural_log_exp" in name]
        used = set()
        for b in self.main_func.blocks:
            for i in b.instructions:
                if isinstance(i, mybir.InstActivation):
                    used.add(i.func)
        if combined and used <= combined[0]:
            # keep positions (set ids) but disqualify competing sets so the
            # greedy placement settles on the one combined set
            tables = [(name, fns if "natural_log_exp" in name else set())
                      for name, fns in tables]
        _bass_rust.insert_act_table_loads(self, tables)

    nc.insert_act_table_loads = types.MethodType(insert_act_table_loads, nc)


def build_kernel(reps=1):
    nc = bacc.Bacc(name="fxp_attn")
    if K_ACTPATCH:
        _patch_act_tables(nc)
    xT_e = nc.declare_dram_parameter("xT", [D, M], F32, isOutput=False)
    wqkvT_e = nc.declare_dram_parameter("wqkvT", [D, 3 * D], F32, isOutput=False)
    wprojT_e = nc.declare_dram_parameter("wprojT", [D, D], F32, isOutput=False)
    bias_e = nc.declare_dram_parameter("bias", [1, D], F32, isOutput=False)
    out_e = nc.declare_dram_parameter("out", [D, NQ], F32, isOutput=True)

    with tile.TileContext(nc) as tc:
        from contextlib import ExitStack
        with ExitStack() as ctx:
            persist = ctx.enter_context(tc.tile_pool(name="persist", bufs=1))

            # ones/4096: S-broadcast matmul directly yields S/4096, so the
            # reciprocal gives rb = 4096/S with no extra scale op.
            ones128 = persist.tile([128, 128], F16, tag="ones128")
            nc.vector.memset(ones128, 1.0 / GRID)
            bias_sb = persist.tile([128, 8], F32, tag="bias")
            nc.sync.dma_start(out=bias_sb, in_=bass.AP(
                tensor=bias_e.ap().tensor, offset=0, ap=[[1, 128], [128, 8]]))

            # persistent fp16 activations
            k_s = [persist.tile([128, M], F16, tag=f"k{s}", name=f"k{s}") for s in range(8)]
            q_s = [persist.tile([128, NQ], F16, tag=f"q{s}", name=f"q{s}") for s in range(8)]
            v_t = [persist.tile([128, D], F16, tag=f"v{t}", name=f"v{t}") for t in range(16)]
            ao_s = [persist.tile([128, NQ], F16, tag=f"ao{s}", name=f"ao{s}")
                    for s in range(8)]

            for _rep in range(reps):
                _run_phases(nc, tc, ones128, bias_sb,
                            k_s, q_s, v_t, ao_s,
                            xT_e, wqkvT_e, wprojT_e, out_e)

    nc.compile()
    return nc


def _run_phases(nc, tc, ones128, bias_sb,
                k_s, q_s, v_t, ao_s,
                xT_e, wqkvT_e, wprojT_e, out_e):
    # ---------------- Phase 0/1: load inputs, qkv matmuls ----------
    with tc.tile_pool(name="ph1", bufs=1) as ph1, \
         tc.tile_pool(name="stg", bufs=2) as stg, \
         tc.tile_pool(name="wstg", bufs=2) as wstg, \
         tc.tile_pool(name="ps1", bufs=8, space="PSUM") as ps1:

        # xT fp16, resident: [128, 8, 2048]
        xT = ph1.tile([128, 8, M], F16, tag="xT")
        for s in range(8):
            st = stg.tile([128, M], F32, tag="stg")
            nc.sync.dma_start(out=st, in_=xT_e.ap()[s * 128:(s + 1) * 128, :])
            nc.vector.tensor_scalar(xT[:, s, :], st, 1.0, None, op0=ALU.mult)

        # w_v fp16 resident: wqkvT cols [2048:3072] -> [128, 8, 1024]
        wv = ph1.tile([128, 8, D], F16, tag="wv")
        for quart in range(4):
            st = stg.tile([128, M], F32, tag="stg")
            st3 = bass.AP(tensor=st.tensor, offset=st.offset,
                          ap=[st.ap[0], [256, 8], [1, 256]])
            nc.sync.dma_start(out=st3, in_=bass.AP(
                tensor=wqkvT_e.ap().tensor,
                offset=2 * D + quart * 256,
                ap=[[3 * D, 128], [3 * D * 128, 8], [1, 256]]))
            nc.vector.tensor_scalar(wv[:, :, quart * 256:(quart + 1) * 256],
                                    st3, 1.0, None, op0=ALU.mult)

        def stream_w(col0, ncols):
            """DMA wqkvT[:, col0:col0+ncols] -> f16 [128, 8, ncols]."""
            st = wstg.tile([128, 8, 128], F32, tag="wstg")
            wf = wstg.tile([128, 8, 128], F16, tag="wstg16")
            nc.sync.dma_start(out=st[:, :, :ncols], in_=bass.AP(
                tensor=wqkvT_e.ap().tensor, offset=col0,
                ap=[[3 * D, 128], [3 * D * 128, 8], [1, ncols]]))
            nc.vector.tensor_scalar(wf[:, :, :ncols], st[:, :, :ncols],
                                    1.0, None, op0=ALU.mult)
            return wf

        # K: feature-major [kv-dim strip s][128, 2048]  (evac on ACT)
        # dt-major inner loops reuse each stationary wf[:, dt, :] across all
        # mc chunks (one LDWEIGHTS per dt instead of per matmul).
        for s in range(8):
            wf = stream_w(D + s * 128, 128)
            pts = [ps1.tile([128, 512], F32, tag="ps1", name=f"k{s}_{mc}")
                   for mc in range(4)]
            for dt in range(8):
                for mc in range(4):
                    nc.tensor.matmul(
                        pts[mc], lhsT=wf[:, dt, :],
                        rhs=xT[:, dt, mc * 512:(mc + 1) * 512],
                        start=(dt == 0), stop=(dt == 7))
            for mc in range(4):
                nc.scalar.copy(k_s[s][:, mc * 512:(mc + 1) * 512], pts[mc])

        # Q: feature-major, tokens 0..1023 of rotated xT  (evac on DVE)
        for s in range(8):
            wf = stream_w(s * 128, 128)
            pts = [ps1.tile([128, 512], F32, tag="ps1", name=f"q{s}_{mc}")
                   for mc in range(2)]
            for dt in range(8):
                for mc in range(2):
                    nc.tensor.matmul(
                        pts[mc], lhsT=wf[:, dt, :],
                        rhs=xT[:, dt, mc * 512:(mc + 1) * 512],
                        start=(dt == 0), stop=(dt == 7))
            for mc in range(2):
                nc.vector.tensor_scalar(q_s[s][:, mc * 512:(mc + 1) * 512],
                                        pts[mc], 1.0, None, op0=ALU.mult)

        # V: token-major [tok strip ts][128, 1024]  (evac on DVE)
        for ts in range(16):
            pts = [ps1.tile([128, 512], F32, tag="ps1", name=f"v{ts}_{cc}")
                   for cc in range(2)]
            for dt in range(8):
                for cc in range(2):
                    nc.tensor.matmul(
                        pts[cc], lhsT=xT[:, dt, ts * 128:(ts + 1) * 128],
                        rhs=wv[:, dt, cc * 512:(cc + 1) * 512],
                        start=(dt == 0), stop=(dt == 7))
            for cc in range(2):
                nc.vector.tensor_scalar(v_t[ts][:, cc * 512:(cc + 1) * 512],
                                        pts[cc], 1.0, None, op0=ALU.mult)

    # ---------------- Phase 2: attention ---------------------------
    with tc.tile_pool(name="expp", bufs=2) as expp, \
         tc.tile_pool(name="attn", bufs=2) as attnp, \
         tc.tile_pool(name="rbp", bufs=2) as rbp, \
         tc.tile_pool(name="dotp", bufs=2, space="PSUM") as dotp, \
         tc.tile_pool(name="sbp", bufs=1, space="PSUM") as sbp, \
         tc.tile_pool(name="avp", bufs=2, space="PSUM") as avp:

        ITERS = [(hp, ch) for hp in range(HP) for ch in range(2)]
        NIT = len(ITERS)

        exp_tiles = {}
        rb_tiles = {}
        af_tiles = {}
        av_tiles = {}

        def issue_dots(i):
            hp, ch = ITERS[i]
            n0 = ch * 512
            eb = expp.tile([128, 2, 16, 512], F16, tag="exp", name=f"exp{i}")
            exp_tiles[i] = eb
            for mt in range(16):
                dt_ps = dotp.tile([128, 2, 512], F32, tag="dt")
                for h in range(2):
                    p0 = h * 64
                    nc.tensor.matmul(
                        dt_ps[:, h, :],
                        lhsT=k_s[hp][p0:p0 + 64, mt * 128:(mt + 1) * 128],
                        rhs=q_s[hp][p0:p0 + 64, n0:n0 + 512],
                        start=True, stop=True,
                        tile_position=(p0, 0))
                nc.scalar.activation(eb[:, :, mt, :], dt_ps, AF.Exp,
                                     scale=0.125)

        def issue_s(i):
            eb = exp_tiles[i]
            rbs = []
            for h in range(2):
                sbc = sbp.tile([128, 512], F32, tag=f"sbc{h}")
                for mt in range(16):
                    nc.tensor.matmul(sbc, lhsT=ones128,
                                     rhs=eb[:, h, mt, :],
                                     start=(mt == 0), stop=(mt == 15))
                # rb4 = 4096/S replicated x4 (unit-stride operand for tt).
                # 1/x via exp(-ln x) on ACT: frees DVE, plenty precise.
                rb4 = rbp.tile([128, 4, 512], F16, tag=f"rb4{h}")
                if K_LNEXP:
                    lnS = rbp.tile([128, 512], F16, tag=f"ln{h}", bufs=1)
                    nc.scalar.activation(lnS, sbc, AF.Ln)
                    with nc.allow_low_precision(reason="1/S fp16 validated"):
                        nc.scalar.activation(rb4[:, 0, :], lnS, AF.Exp,
                                             scale=-1.0)
                else:
                    with nc.allow_low_precision(reason="1/S fp16 validated"):
                        nc.vector.reciprocal(rb4[:, 0, :], sbc)
                for j in range(1, 4):
                    nc.gpsimd.tensor_copy(rb4[:, j, :], rb4[:, 0, :])
                rbs.append(rb4)
            rb_tiles[i] = rbs

        def issue_quant_av(i):
            hp, ch = ITERS[i]
            n0 = ch * 512
            eb = exp_tiles.pop(i)
            rbs = rb_tiles.pop(i)
            av = avp.tile([128, 512], F32, tag="av")
            for g in range(4):
                afs = []
                for h in range(2):
                    y = attnp.tile([128, 4, 512], F16, tag="y", bufs=1)
                    nc.vector.tensor_tensor(
                        y, eb[:, h, 4 * g:4 * g + 4, :], rbs[h], op=ALU.mult)
                    ai = attnp.tile([128, 4, 512], I16, tag="ai", bufs=1)
                    nc.vector.tensor_scalar(ai, y, -0.499, None, op0=ALU.add)
                    af = attnp.tile([128, 4, 512], F16, tag="af",
                                    name=f"af{i}_{g}_{h}")
                    nc.vector.tensor_scalar(af, ai, 1.0, None, op0=ALU.mult)
                    afs.append(af)
                for sub in range(4):
                    mt = 4 * g + sub
                    for h in range(2):
                        p0 = h * 64
                        nc.tensor.matmul(
                            av[p0:p0 + 64, :],
                            lhsT=v_t[mt][:, (2 * hp + h) * 64:
                                         (2 * hp + h + 1) * 64],
                            rhs=afs[h][:, sub, :],
                            start=(mt == 0), stop=(mt == 15),
                            tile_position=(0, p0))
            # evacuate: natural units = grid/4096
            nc.vector.tensor_scalar(ao_s[hp][:, n0:n0 + 512], av,
                                    1.0 / GRID, None, op0=ALU.mult)

        # software pipeline, PE issue order per step: S(i-1); dots(i); av(i-1)
        for i in range(NIT + 1):
            if K_SORDER:
                if i >= 1:
                    issue_s(i - 1)
                if i < NIT:
                    issue_dots(i)
                if i >= 1:
                    issue_quant_av(i - 1)
            else:
                if i < NIT:
                    issue_dots(i)
                if i >= 1:
                    issue_s(i - 1)
                    issue_quant_av(i - 1)

    # ---------------- Phase 3: projection --------------------------
    with tc.tile_pool(name="ps3", bufs=4, space="PSUM") as ps3, \
         tc.tile_pool(name="wpp", bufs=1) as wpp, \
         tc.tile_pool(name="wstg3", bufs=2) as wstg3, \
         tc.tile_pool(name="outp", bufs=2) as outp:
        wp_s = []
        for s in range(8):
            st = wstg3.tile([128, D], F32, tag="stg3")
            nc.sync.dma_start(out=st, in_=wprojT_e.ap()[s * 128:(s + 1) * 128, :])
            wp = wpp.tile([128, D], F16, tag=f"wp{s}")
            nc.vector.tensor_scalar(wp, st, 1.0, None, op0=ALU.mult)
            wp_s.append(wp)
        for ds in range(8):
            pts = [ps3.tile([128, 512], F32, tag="ps3", name=f"p3_{ds}_{ch}")
                   for ch in range(2)]
            for es in range(8):
                for ch in range(2):
                    nc.tensor.matmul(
                        pts[ch], lhsT=wp_s[es][:, ds * 128:(ds + 1) * 128],
                        rhs=ao_s[es][:, ch * 512:(ch + 1) * 512],
                        start=(es == 0), stop=(es == 7))
            for ch in range(2):
                ot = outp.tile([128, 512], F32, tag="ot")
                nc.vector.tensor_scalar(ot, pts[ch], bias_sb[:, ds:ds + 1],
                                        None, op0=ALU.add)
                nc.sync.dma_start(
                    out=out_e.ap()[ds * 128:(ds + 1) * 128,
                                   ch * 512:(ch + 1) * 512],
                    in_=ot)


def _get_nc():
    global _CACHED_NC
    if _CACHED_NC is None:
        _CACHED_NC = build_kernel()
    return _CACHED_NC


def prep(inputs):
    """Build (nc, in_maps) for the 8 cores from full inputs."""
    x, w_qkv, w_proj, b_proj = (inputs["x"], inputs["w_qkv"],
                                inputs["w_proj"], inputs["b_proj"])
    nc = _get_nc()
    wqkvT = np.ascontiguousarray(w_qkv.astype(np.float32).T)
    wprojT = np.ascontiguousarray(w_proj.astype(np.float32).T)
    bias = b_proj.astype(np.float32).reshape(1, D)

    in_maps = []
    for c in range(8):
        b, t = c // 2, c % 2
        xb = x[b].astype(np.float32)
        xrot = np.concatenate([xb[t * NQ:], xb[:t * NQ]], axis=0)
        in_maps.append({
            "xT": np.ascontiguousarray(xrot.T),
            "wqkvT": wqkvT,
            "wprojT": wprojT,
            "bias": bias,
        })
    return nc, in_maps


def kernel(x, w_qkv, w_proj, b_proj, **_):
    B, N, Dm = x.shape
    assert (B, N, Dm) == (4, 2048, 1024)
    nc, in_maps = prep({"x": x, "w_qkv": w_qkv, "w_proj": w_proj,
                        "b_proj": b_proj})

    res = run_bass_kernel_spmd(nc, in_maps, list(range(8)))
    global LAST_RESULT
    LAST_RESULT = res
    out = np.empty((B, N, Dm), dtype=np.float32)
    for c in range(8):
        b, t = c // 2, c % 2
        out[b, t * NQ:(t + 1) * NQ, :] = res.results[c]["out"].T
    return out
